# revision 3
# baseline (speedup 1.0000x reference)
"""DiffTreeInterpreter scatter-coalesce kernel for 8 Trainium2 cores.

Data-parallel over batch B=32: core c owns batches [4c, 4c+4). All
scatter-adds are device-local. Host work is limited to sharding-style
index prep: bucketing entries by (batch, role-block), and shipping
bit-exact *copies* of per-entry weights (arg_weights / op_dist rows
selected by index) alongside the value stream. All arithmetic
(weight products, value scaling, coalesce sums, stream combine)
happens on the NeuronCores.

Math (see reference): with H = R/2, each entry n (b, l, r, v=mem[n],
w=arg_weights[b,l]) contributes to out[b] at up to 3 bins:
  bin r>>1   with weight op0[b]*w0 if r even, op1[b]*w1 if r odd and r!=1
  bin 2r     with weight op2[b]*w2 (only r < H)
  bin 2r+1   with weight op2[b]*w3 (only r < H)
plus out[b,1] += op2[b]*root_filler[b].
(The reference's pad-mask is a no-op on values: masked rows are all-zero.)

Device algorithm per core: entries are bucketed into 128-entry tiles
aligned to role windows. For each tile, build u-scaled one-hot
matrices O[p, j] = u[p] * (iota[j] == r_rel[p]) on the Vector engine
(one fused tensor_scalar each), and matmul O^T @ V on the PE into a
PSUM block of 128 output bins. PSUM blocks drain to SBUF staging and
DMA out contiguously.
"""

import sys

if "/opt/trn_rl_repo" not in sys.path:
    sys.path.insert(0, "/opt/trn_rl_repo")

import numpy as np

B, L, F, R = 32, 128, 128, 4096
H = R >> 1
N = 262144
NCORES = 8
BPC = B // NCORES  # batches per core

P = 128  # partitions / tile entry count / bin-block size

# Static schedule: per batch,
#  lower half (r < 2048): 32 role-blocks of 64 r-values -> bins [128k, 128k+128)
#     via cons streams (2r, 2r+1); capacity LOW_CAP tiles each.
#  upper half (r >= 2048): 8 role-blocks of 256 r-values, car/cdr stream only;
#     capacity UP_CAP tiles each.
#  car/cdr stream of lower-half r also feeds S1 blocks (bins r>>1).
LOW_CAP = 2  # tiles per (batch, 64-r block), holds <= 256 entries (data max 161+1)
UP_CAP = 5  # tiles per (batch, 256-r block), holds <= 640 entries (data max 575)
TILES_PER_BATCH = 32 * LOW_CAP + 8 * UP_CAP  # 104
NT = BPC * TILES_PER_BATCH  # tiles per core (416)

# meta channels
MC_R1, MC_WA, MC_OPA, MC_R23, MC_WB, MC_WC, MC_OP2, MC_PAD = range(8)
NMC = 8

_PROG_CACHE = {}


def _tile_base(b, lower, blk):
    """Global tile index of slot 0 of a block within a core's stream."""
    base = b * TILES_PER_BATCH
    if lower:
        return base + blk * LOW_CAP
    return base + 32 * LOW_CAP + blk * UP_CAP


def _build_program():
    import concourse.bacc as bacc
    import concourse.mybir as mybir
    import concourse.tile as tile

    fp32 = mybir.dt.float32
    EQ = mybir.AluOpType.is_equal
    MUL = mybir.AluOpType.mult
    ADD = mybir.AluOpType.add

    nc = bacc.Bacc(None, target_bir_lowering=False)
    vals = nc.dram_tensor("vals", [NT * P, F], fp32, kind="ExternalInput")
    meta = nc.dram_tensor("meta", [NT, P, NMC], fp32, kind="ExternalInput")
    iota = nc.dram_tensor("iota", [P, P], fp32, kind="ExternalInput")
    out = nc.dram_tensor("out", [BPC, R, F], fp32, kind="ExternalOutput")

    vals_t = vals.rearrange("(t p) f -> t p f", p=P)

    with tile.TileContext(nc) as tc:
        with tc.tile_pool(name="const", bufs=1) as cpool, \
             tc.tile_pool(name="metap", bufs=2) as mpool, \
             tc.tile_pool(name="useq", bufs=2) as upool, \
             tc.tile_pool(name="vload", bufs=6) as vpool, \
             tc.tile_pool(name="ohot", bufs=6) as opool, \
             tc.tile_pool(name="stage", bufs=14) as spool, \
             tc.tile_pool(name="ps1", bufs=2, space="PSUM") as ps1pool, \
             tc.tile_pool(name="ps23", bufs=2, space="PSUM") as ps23pool:

            io_t = cpool.tile([P, P], fp32)
            nc.sync.dma_start(out=io_t[:], in_=iota[:])

            for b in range(BPC):
                # whole batch's metadata: [104, 128, 8] -> SBUF [128, 104, 8]
                m = mpool.tile([P, TILES_PER_BATCH, NMC], fp32)
                nc.sync.dma_start(
                    out=m[:],
                    in_=meta[b * TILES_PER_BATCH:(b + 1) * TILES_PER_BATCH]
                    .rearrange("t p c -> p t c"),
                )
                # u slabs [128, 104]: u1 = wA*opA, u2 = wB*op2, u3 = wC*op2
                u1 = upool.tile([P, TILES_PER_BATCH], fp32, tag="u1")
                u2 = upool.tile([P, TILES_PER_BATCH], fp32, tag="u2")
                u3 = upool.tile([P, TILES_PER_BATCH], fp32, tag="u3")
                nc.vector.tensor_tensor(
                    out=u1[:], in0=m[:, :, MC_WA], in1=m[:, :, MC_OPA], op=MUL)
                nc.vector.tensor_tensor(
                    out=u2[:], in0=m[:, :, MC_WB], in1=m[:, :, MC_OP2], op=MUL)
                nc.vector.tensor_tensor(
                    out=u3[:], in0=m[:, :, MC_WC], in1=m[:, :, MC_OP2], op=MUL)

                stage = {}

                def do_tile(t, ps1, s1_start, s1_stop, ps23=None,
                            s23_start=False, s23_stop=False):
                    tg = b * TILES_PER_BATCH + t
                    v = vpool.tile([P, F], fp32, tag="v")
                    nc.sync.dma_start(out=v[:], in_=vals_t[tg])
                    # car/cdr one-hot: O1[p, j] = u1[p] * (iota[p,j] == r1_rel[p])
                    o1 = opool.tile([P, P], fp32, tag="o1")
                    nc.vector.tensor_scalar(
                        out=o1[:], in0=io_t[:],
                        scalar1=m[:, t, MC_R1:MC_R1 + 1],
                        scalar2=u1[:, t:t + 1],
                        op0=EQ, op1=MUL)
                    nc.tensor.matmul(out=ps1[:], lhsT=o1[:], rhs=v[:],
                                     start=s1_start, stop=s1_stop)
                    if ps23 is not None:
                        # cons one-hots, interleaved into even/odd columns
                        o23 = opool.tile([P, 64, 2], fp32, tag="o23")
                        nc.vector.tensor_scalar(
                            out=o23[:, :, 0], in0=io_t[:, 0:64],
                            scalar1=m[:, t, MC_R23:MC_R23 + 1],
                            scalar2=u2[:, t:t + 1],
                            op0=EQ, op1=MUL)
                        nc.vector.tensor_scalar(
                            out=o23[:, :, 1], in0=io_t[:, 0:64],
                            scalar1=m[:, t, MC_R23:MC_R23 + 1],
                            scalar2=u3[:, t:t + 1],
                            op0=EQ, op1=MUL)
                        nc.tensor.matmul(
                            out=ps23[:],
                            lhsT=o23[:].rearrange("p a t -> p (a t)"),
                            rhs=v[:], start=s23_start, stop=s23_stop)

                for bk1 in range(16):
                    ps1 = ps1pool.tile([P, F], fp32, tag="ps1")
                    if bk1 < 8:
                        nmm = 4 * LOW_CAP
                        i1 = 0
                        for j in range(4):
                            k = 4 * bk1 + j
                            ps23 = ps23pool.tile([P, F], fp32, tag="ps23")
                            for s in range(LOW_CAP):
                                t = _tile_base(0, True, k) + s
                                do_tile(t, ps1, i1 == 0, i1 == nmm - 1,
                                        ps23, s == 0, s == LOW_CAP - 1)
                                i1 += 1
                            st = spool.tile([P, F], fp32, tag="st")
                            nc.scalar.copy(out=st[:], in_=ps23[:])
                            stage[k] = st
                            if k >= 16:
                                nc.sync.dma_start(
                                    out=out[b, k * P:(k + 1) * P, :], in_=st[:])
                    else:
                        ub = bk1 - 8
                        for s in range(UP_CAP):
                            t = _tile_base(0, False, ub) + s
                            do_tile(t, ps1, s == 0, s == UP_CAP - 1)
                    # car/cdr drain: bins [128*bk1, +128) -> add into stage
                    st = stage[bk1]
                    nc.vector.tensor_tensor(
                        out=st[:], in0=st[:], in1=ps1[:], op=ADD)
                    nc.sync.dma_start(
                        out=out[b, bk1 * P:(bk1 + 1) * P, :], in_=st[:])

    nc.compile()
    return nc


def _pack_inputs(mem_values, arg_weights, root_filler, op_dist,
                 batch_idx, slot_idx, role_idx):
    """Host-side sharding/packing. Index selection and copies only."""
    mem_values = np.ascontiguousarray(mem_values, dtype=np.float32)
    arg_weights = np.asarray(arg_weights, dtype=np.float32)
    root_filler = np.asarray(root_filler, dtype=np.float32)
    op_dist = np.asarray(op_dist, dtype=np.float32)
    batch_idx = np.asarray(batch_idx, dtype=np.int64)
    slot_idx = np.asarray(slot_idx, dtype=np.int64)
    role_idx = np.asarray(role_idx, dtype=np.int64)

    # per-entry selected copies (pure gathers, no arithmetic)
    w = arg_weights[batch_idx, slot_idx]  # [N, 4] copies
    r = role_idx
    even = (r & 1) == 0
    wA = np.where(even, w[:, 0], np.where(r != 1, w[:, 1], 0.0)).astype(np.float32)
    opA = np.where(even, op_dist[batch_idx, 0], op_dist[batch_idx, 1]).astype(np.float32)
    lower = r < H
    wB = np.where(lower, w[:, 2], 0.0).astype(np.float32)
    wC = np.where(lower, w[:, 3], 0.0).astype(np.float32)
    op2c = op_dist[batch_idx, 2].astype(np.float32)

    # block id within batch-stream: lower blocks 0..31 (64 r each),
    # upper blocks 32..39 (256 r each)
    blk = np.where(lower, r >> 6, 32 + ((r - H) >> 8))
    # capacity slots per block
    cap_slots = np.concatenate([
        np.full(32, LOW_CAP * P, np.int64), np.full(8, UP_CAP * P, np.int64)])
    blk_slot0 = np.concatenate([[0], np.cumsum(cap_slots)])[:-1]  # [40]

    in_maps = []
    for c in range(NCORES):
        vals_s = np.zeros((NT * P, F), np.float32)
        meta_s = np.zeros((NT, P, NMC), np.float32)
        meta_s[:, :, MC_R1] = -1.0
        meta_s[:, :, MC_R23] = -1.0
        for bb in range(BPC):
            b = c * BPC + bb
            sel = np.nonzero(batch_idx == b)[0]
            gb = blk[sel]
            order = np.argsort(gb, kind="stable")
            sel = sel[order]
            gb = gb[order]
            counts = np.bincount(gb, minlength=40)
            if (counts[:32] > LOW_CAP * P - 1).any() or (counts[32:] > UP_CAP * P).any():
                raise RuntimeError(
                    "static schedule capacity exceeded; "
                    f"counts max lower={counts[:32].max()} upper={counts[32:].max()}")
            first = np.concatenate([[0], np.cumsum(counts)])[:-1]
            pos_in_blk = np.arange(sel.size) - first[gb]
            slot = blk_slot0[gb] + pos_in_blk  # slot within the batch stream
            slot += bb * TILES_PER_BATCH * P
            vals_s[slot] = mem_values[sel]
            tix, pix = slot // P, slot % P
            rr = role_idx[sel]
            meta_s[tix, pix, MC_R1] = ((rr >> 1) & 127).astype(np.float32)
            meta_s[tix, pix, MC_WA] = wA[sel]
            meta_s[tix, pix, MC_OPA] = opA[sel]
            meta_s[tix, pix, MC_R23] = np.where(rr < H, (rr & 63), -1).astype(np.float32)
            meta_s[tix, pix, MC_WB] = wB[sel]
            meta_s[tix, pix, MC_WC] = wC[sel]
            meta_s[tix, pix, MC_OP2] = op2c[sel]
            # synthetic root entry -> bin 1 == 2*0+1 (block 0, odd cons slot)
            rslot = bb * TILES_PER_BATCH * P + counts[0]
            vals_s[rslot] = root_filler[b]
            ti, pi = rslot // P, rslot % P
            meta_s[ti, pi, MC_R1] = -1.0
            meta_s[ti, pi, MC_R23] = 0.0
            meta_s[ti, pi, MC_WC] = 1.0
            meta_s[ti, pi, MC_OP2] = op_dist[b, 2]
        in_maps.append({
            "vals": vals_s,
            "meta": meta_s,
            "iota": np.broadcast_to(
                np.arange(P, dtype=np.float32), (P, P)).copy(),
        })
    return in_maps


def kernel(**inputs):
    from concourse.bass_utils import run_bass_kernel_spmd

    in_maps = _pack_inputs(**inputs)
    if "nc" not in _PROG_CACHE:
        _PROG_CACHE["nc"] = _build_program()
    nc = _PROG_CACHE["nc"]
    res = run_bass_kernel_spmd(nc, in_maps, list(range(NCORES)))
    return np.concatenate([res.results[c]["out"] for c in range(NCORES)], axis=0)
